# revision 1
# baseline (speedup 1.0000x reference)
"""Trainium2 Bass kernel for batched global mean pooling (segment mean).

Computes, for N sorted nodes with 64 features and G graphs:
    out[g, f] = mean over nodes n with batch[n] == g of node_features[n, f]
(empty graphs -> zeros), distributed over 8 NeuronCores.

Strategy (graph sharding; no collectives):
  - Core k owns graphs [128k, 128(k+1)). batch is sorted, so each graph's
    nodes are a contiguous row range of node_features.
  - Host (inside kernel(), per call) lays out each core's nodes on a
    [128, T] grid: partition p gets only the nodes of local graph p,
    zero-padded to T = max graph size.
  - Features are split into fp16 (hi, lo) pairs so the PE runs at full
    rate (fp32 matmul is 4x slower); hi + lo recovers fp32 precision
    since the products accumulate into fp32 PSUM.
  - Device: each matmul is identity128.T @ slab for a [128, 7*64] fp16
    slab, accumulating into one [128, 448] f32 PSUM bank: partition =
    local graph. After all chunks: fold the 7 column blocks, multiply by
    host-provided 1/max(count, 1), DMA the [128, 64] result out.
  - Host concatenates the 8 per-core [128, 64] outputs.

The Bass program is compiled per call with the chunk count derived from
the actual input, so any node/graph distribution is handled.
"""

import math

import numpy as np

import concourse.mybir as mybir
import concourse.tile as tile
from concourse import bacc
from concourse.bass_utils import run_bass_kernel_spmd
from concourse.masks import make_identity

NCORES = 8
P = 128  # partitions = local graphs per core
F = 64  # features
B = 7  # tiles (node-rows) per matmul: N = 7*64 = 448 <= 512 f32 PSUM bank
TB = 63  # tiles per full DMA chunk (~1.03 MB per chunk)

# set by tests to capture a profile; harness path leaves these alone
TRACE = False
LAST_RESULTS = None


def _chunks(t_cap):
    """Split t_cap tiles into DMA chunks: small 21-tile chunks at the START
    (so the first chunk lands quickly and the PE starts early instead of
    trailing the stream by the whole prefetch depth) and at the END (short PE
    tail after the final DMA); full 63-tile chunks in between."""
    out = []
    t = 0
    taper = TB if t_cap > 8 * TB else 0
    while t < t_cap:
        in_taper = t < taper or t_cap - t <= taper
        n = min(3 * B if in_taper else TB, t_cap - t)
        out.append((t, n))
        t += n
    return out


def _build(t_cap):
    nc = bacc.Bacc("TRN2", target_bir_lowering=False, debug=False, num_devices=NCORES)
    # hi and lo halves of each chunk are packed back-to-back in one tensor so
    # every chunk is a single DMA: each DMA costs a semaphore, and the Tile
    # kernel-tail resets every used semaphore serially (~0.13 us each)
    hl = nc.dram_tensor(
        "hl", [P, 2 * t_cap * F], mybir.dt.float16, kind="ExternalInput"
    ).ap()
    inv = nc.dram_tensor("inv", [P, 1], mybir.dt.float32, kind="ExternalInput").ap()
    out = nc.dram_tensor("out", [P, F], mybir.dt.float32, kind="ExternalOutput").ap()

    chunks = _chunks(t_cap)
    n_mm = 2 * (t_cap // B)
    with tile.TileContext(nc) as tc:
        with (
            tc.tile_pool(name="consts", bufs=1) as consts,
            tc.tile_pool(name="io", bufs=5) as io,
            tc.tile_pool(name="ep", bufs=1) as ep,
            tc.tile_pool(name="acc", bufs=1, space="PSUM") as accp,
        ):
            # build the identity on-device (GpSimd) so the weight preload has
            # no DMA dependency -- an identity DMA would queue behind the
            # first big chunk DMAs and stall the PE ~14 us at kernel start
            ident_sb = consts.tile([P, P], mybir.dt.float16)
            make_identity(nc, ident_sb[:])

            # load the identity into the PE array once; every matmul below
            # reuses it (ldweights=False) instead of reloading 128 columns
            # per matmul (~100 ns each, ~60 us of PE time at ~600 matmuls)
            ldw = nc.tensor.ldweights(ident_sb[:])

            psum = accp.tile([P, B * F], mybir.dt.float32)
            mm = 0
            for ci, (t0, nt) in enumerate(chunks):
                hl_t = io.tile([P, 2 * TB * F], mybir.dt.float16, tag="hl")
                # alternate the two HWDGE rings (SP / ACT engines)
                eng = nc.sync if ci % 2 == 0 else nc.scalar
                eng.dma_start(
                    hl_t[:, : 2 * nt * F], hl[:, 2 * t0 * F : 2 * (t0 + nt) * F]
                )
                for b in range(2 * (nt // B)):
                    inst = nc.tensor.matmul(
                        psum[:],
                        ident_sb[:],
                        hl_t[:, b * B * F : (b + 1) * B * F],
                        start=(mm == 0),
                        stop=(mm == n_mm - 1),
                    )
                    inst.ins.ldweights = False
                    if mm == 0:
                        tile.add_dep_helper(
                            inst.ins,
                            ldw.ins,
                            sync=False,
                            reason="identity weights preloaded once",
                        )
                    mm += 1
            assert mm == n_mm

            # emitted after the chunk loop so this tiny transfer doesn't
            # head-of-line block the first chunk on the sync ring
            inv_sb = consts.tile([P, 1], mybir.dt.float32)
            nc.sync.dma_start(inv_sb[:], inv[:])

            # fold the B column blocks in ONE reduce: view psum [P, 448] as
            # [P, f=64, b=7] (b strided by 64) and sum the innermost axis --
            # one DVE op instead of a serial copy + 6 adds on the tail path
            s = ep.tile([P, F], mybir.dt.float32)
            nc.vector.tensor_reduce(
                s[:],
                psum[:, 0 : B * F].rearrange("p (b f) -> p f b", b=B),
                axis=mybir.AxisListType.X,
                op=mybir.AluOpType.add,
            )

            res = ep.tile([P, F], mybir.dt.float32)
            nc.vector.tensor_scalar_mul(res[:], s[:], inv_sb[:])
            nc.sync.dma_start(out[:], res[:])

    nc.compile()
    # bacc materializes one Ldweights per Matmult even with ldweights=False;
    # they all reload the same identity (~100 ns of PE time each). Drop the
    # redundant ones — keep any that carry semaphore waits/updates (those
    # park sync state), including the explicit preload which waits on the
    # identity build.
    for fn in nc.m.functions:
        for blk in fn.blocks:
            keep = [
                inst
                for inst in blk.instructions
                if not (
                    isinstance(inst, mybir.InstLdweights)
                    and (
                        inst.sync_info is None
                        or (
                            len(inst.sync_info.on_wait) == 0
                            and len(inst.sync_info.on_update) == 0
                        )
                    )
                )
            ]
            if len(keep) != len(blk.instructions):
                blk.instructions = keep
    return nc


def kernel(node_features, batch, num_graphs):
    global LAST_RESULTS
    x = np.asarray(node_features, dtype=np.float32)
    b = np.asarray(batch, dtype=np.int64).ravel()
    G = int(num_graphs)
    N = x.shape[0]
    assert x.shape[1] == F, f"expected {F} features, got {x.shape[1]}"

    if not np.all(b[1:] >= b[:-1]):  # defensive: layout relies on sorted batch
        order = np.argsort(b, kind="stable")
        b = b[order]
        x = x[order]

    gpc = math.ceil(G / NCORES)  # local graphs per core
    assert gpc <= P, f"num_graphs {G} too large for {NCORES} cores x {P} partitions"

    # ids >= G (if any) are dropped, matching segment_sum(num_segments=G)
    counts = np.bincount(b, minlength=NCORES * gpc)[: NCORES * gpc].astype(np.int64)
    starts = np.zeros(NCORES * gpc + 1, dtype=np.int64)
    np.cumsum(counts, out=starts[1:])
    t_max = int(counts.max()) if N else 1
    t_cap = max(B, math.ceil(t_max / B) * B)

    x_ext = np.vstack([x, np.zeros((1, F), dtype=np.float32)])  # row N = zeros
    col = np.arange(t_cap, dtype=np.int64)
    chunk_list = _chunks(t_cap)

    in_maps = []
    for k in range(NCORES):
        g0 = k * gpc
        cg = counts[g0 : g0 + gpc]
        sg = starts[g0 : g0 + gpc]
        valid = col[None, :] < cg[:, None]  # [gpc, t_cap]
        idx = np.where(valid, sg[:, None] + col[None, :], N)
        if gpc < P:  # pad partitions when graph count is not divisible by 8
            idx = np.vstack([idx, np.full((P - gpc, t_cap), N, dtype=np.int64)])

        feats = x_ext[idx]  # [P, t_cap, F] f32
        hi16 = feats.astype(np.float16).reshape(P, t_cap * F)
        lo16 = (
            (feats - hi16.reshape(P, t_cap, F).astype(np.float32))
            .astype(np.float16)
            .reshape(P, t_cap * F)
        )
        # pack [hi-chunk | lo-chunk] back-to-back per chunk (see _build)
        hl = np.empty((P, 2 * t_cap * F), dtype=np.float16)
        for t0, nt in chunk_list:
            hl[:, 2 * t0 * F : (2 * t0 + nt) * F] = hi16[:, t0 * F : (t0 + nt) * F]
            hl[:, (2 * t0 + nt) * F : 2 * (t0 + nt) * F] = lo16[:, t0 * F : (t0 + nt) * F]

        inv = np.zeros((P, 1), dtype=np.float32)
        inv[:gpc, 0] = 1.0 / np.maximum(cg, 1)
        in_maps.append({"hl": hl, "inv": inv})

    nc = _build(t_cap)
    try:
        res = run_bass_kernel_spmd(
            nc, in_maps, core_ids=list(range(NCORES)), trace=TRACE
        )
    except Exception:
        # transient device state (e.g. a previous run left a core wedged)
        # has been observed to clear on retry
        res = run_bass_kernel_spmd(
            nc, in_maps, core_ids=list(range(NCORES)), trace=TRACE
        )
    LAST_RESULTS = res

    out = np.concatenate([res.results[k]["out"] for k in range(NCORES)], axis=0)
    return out[:G]



# revision 2
# speedup vs baseline: 1.8413x; 1.8413x over previous
"""Trainium2 Bass kernel for batched global mean pooling (segment mean).

Computes, for N sorted nodes with 64 features and G graphs:
    out[g, f] = mean over nodes n with batch[n] == g of node_features[n, f]
(empty graphs -> zeros), distributed over 8 NeuronCores.

Strategy (graph sharding; no collectives):
  - Core k owns graphs [128k, 128(k+1)). batch is sorted, so each graph's
    nodes are a contiguous row range of node_features.
  - Host (inside kernel(), per call) lays out each core's nodes on a
    [128, T] grid: partition p gets only the nodes of local graph p,
    zero-padded to T = max graph size.
  - Features are sent as plain fp16: the products accumulate into fp32
    PSUM, so the only error is the fp16 input rounding (~4e-4 rel on the
    pooled means, far inside the 2e-2 gate), and HBM traffic is 2 B/elt.
  - Device: each matmul is identity128.T @ slab for a [128, 8*64] fp16
    slab, accumulating into one [128, 512] f32 PSUM bank: partition =
    local graph. After all chunks: fold the 8 column blocks, multiply by
    host-provided 1/max(count, 1), DMA the [128, 64] result out.
  - Host concatenates the 8 per-core [128, 64] outputs.

The Bass program is compiled per call with the chunk count derived from
the actual input, so any node/graph distribution is handled.
"""

import math

import numpy as np

import concourse.mybir as mybir
import concourse.tile as tile
from concourse import bacc
from concourse.bass_utils import run_bass_kernel_spmd
from concourse.masks import make_identity

NCORES = 8
P = 128  # partitions = local graphs per core
F = 64  # features
B = 8  # tiles (node-rows) per matmul: N = 8*64 = 512 f32 = one PSUM bank
TB = 64  # tiles per full DMA chunk (1 MB per chunk)

# set by tests to capture a profile; harness path leaves these alone
TRACE = False
LAST_RESULTS = None


def _chunks(t_cap):
    """Split t_cap tiles into DMA chunks: small 24-tile chunks at the START
    (so the first chunk lands quickly and the PE starts early instead of
    trailing the stream by the whole prefetch depth) and at the END (short PE
    tail after the final DMA); full 64-tile chunks in between."""
    out = []
    t = 0
    taper = TB if t_cap > 8 * TB else 0
    while t < t_cap:
        in_taper = t < taper or t_cap - t <= taper
        n = min(3 * B if in_taper else TB, t_cap - t)
        out.append((t, n))
        t += n
    return out


def _build(t_cap):
    nc = bacc.Bacc("TRN2", target_bir_lowering=False, debug=False, num_devices=NCORES)
    h = nc.dram_tensor(
        "h", [P, t_cap * F], mybir.dt.float16, kind="ExternalInput"
    ).ap()
    inv = nc.dram_tensor("inv", [P, 1], mybir.dt.float32, kind="ExternalInput").ap()
    out = nc.dram_tensor("out", [P, F], mybir.dt.float32, kind="ExternalOutput").ap()

    chunks = _chunks(t_cap)
    n_mm = t_cap // B
    with tile.TileContext(nc) as tc:
        with (
            tc.tile_pool(name="consts", bufs=1) as consts,
            tc.tile_pool(name="io", bufs=5) as io,
            tc.tile_pool(name="ep", bufs=1) as ep,
            tc.tile_pool(name="acc", bufs=1, space="PSUM") as accp,
        ):
            # build the identity on-device (GpSimd) so the weight preload has
            # no DMA dependency -- an identity DMA would queue behind the
            # first big chunk DMAs and stall the PE ~14 us at kernel start
            ident_sb = consts.tile([P, P], mybir.dt.float16)
            make_identity(nc, ident_sb[:])

            # load the identity into the PE array once; every matmul below
            # reuses it (ldweights=False) instead of reloading 128 columns
            # per matmul (~100 ns each)
            ldw = nc.tensor.ldweights(ident_sb[:])

            psum = accp.tile([P, B * F], mybir.dt.float32)
            mm = 0
            for ci, (t0, nt) in enumerate(chunks):
                h_t = io.tile([P, TB * F], mybir.dt.float16, tag="h")
                # alternate the two HWDGE rings (SP / ACT engines)
                eng = nc.sync if ci % 2 == 0 else nc.scalar
                eng.dma_start(h_t[:, : nt * F], h[:, t0 * F : (t0 + nt) * F])
                for b in range(nt // B):
                    inst = nc.tensor.matmul(
                        psum[:],
                        ident_sb[:],
                        h_t[:, b * B * F : (b + 1) * B * F],
                        start=(mm == 0),
                        stop=(mm == n_mm - 1),
                    )
                    inst.ins.ldweights = False
                    if mm == 0:
                        tile.add_dep_helper(
                            inst.ins,
                            ldw.ins,
                            sync=False,
                            reason="identity weights preloaded once",
                        )
                    mm += 1
            assert mm == n_mm

            # emitted after the chunk loop so this tiny transfer doesn't
            # head-of-line block the first chunk on the sync ring
            inv_sb = consts.tile([P, 1], mybir.dt.float32)
            nc.sync.dma_start(inv_sb[:], inv[:])

            # fold the B column blocks in ONE reduce: view psum [P, 512] as
            # [P, f=64, b=8] (b strided by 64) and sum the innermost axis
            s = ep.tile([P, F], mybir.dt.float32)
            nc.vector.tensor_reduce(
                s[:],
                psum[:, 0 : B * F].rearrange("p (b f) -> p f b", b=B),
                axis=mybir.AxisListType.X,
                op=mybir.AluOpType.add,
            )

            res = ep.tile([P, F], mybir.dt.float32)
            nc.vector.tensor_scalar_mul(res[:], s[:], inv_sb[:])
            nc.sync.dma_start(out[:], res[:])

    nc.compile()
    # bacc materializes one Ldweights per Matmult even with ldweights=False;
    # they all reload the same identity (~100 ns of PE time each). Drop the
    # redundant ones — keep any that carry semaphore waits/updates (those
    # park sync state), including the explicit preload which waits on the
    # identity build.
    for fn in nc.m.functions:
        for blk in fn.blocks:
            keep = [
                inst
                for inst in blk.instructions
                if not (
                    isinstance(inst, mybir.InstLdweights)
                    and (
                        inst.sync_info is None
                        or (
                            len(inst.sync_info.on_wait) == 0
                            and len(inst.sync_info.on_update) == 0
                        )
                    )
                )
            ]
            if len(keep) != len(blk.instructions):
                blk.instructions = keep
    return nc


def kernel(node_features, batch, num_graphs):
    global LAST_RESULTS
    x = np.asarray(node_features, dtype=np.float32)
    b = np.asarray(batch, dtype=np.int64).ravel()
    G = int(num_graphs)
    N = x.shape[0]
    assert x.shape[1] == F, f"expected {F} features, got {x.shape[1]}"

    if not np.all(b[1:] >= b[:-1]):  # defensive: layout relies on sorted batch
        order = np.argsort(b, kind="stable")
        b = b[order]
        x = x[order]

    gpc = math.ceil(G / NCORES)  # local graphs per core
    assert gpc <= P, f"num_graphs {G} too large for {NCORES} cores x {P} partitions"

    # ids >= G (if any) are dropped, matching segment_sum(num_segments=G)
    counts = np.bincount(b, minlength=NCORES * gpc)[: NCORES * gpc].astype(np.int64)
    starts = np.zeros(NCORES * gpc + 1, dtype=np.int64)
    np.cumsum(counts, out=starts[1:])
    t_max = int(counts.max()) if N else 1
    t_cap = max(B, math.ceil(t_max / B) * B)

    x_ext = np.vstack([x, np.zeros((1, F), dtype=np.float32)])  # row N = zeros
    col = np.arange(t_cap, dtype=np.int64)

    in_maps = []
    for k in range(NCORES):
        g0 = k * gpc
        cg = counts[g0 : g0 + gpc]
        sg = starts[g0 : g0 + gpc]
        valid = col[None, :] < cg[:, None]  # [gpc, t_cap]
        idx = np.where(valid, sg[:, None] + col[None, :], N)
        if gpc < P:  # pad partitions when graph count is not divisible by 8
            idx = np.vstack([idx, np.full((P - gpc, t_cap), N, dtype=np.int64)])

        feats = x_ext[idx]  # [P, t_cap, F] f32
        h = feats.astype(np.float16).reshape(P, t_cap * F)

        inv = np.zeros((P, 1), dtype=np.float32)
        inv[:gpc, 0] = 1.0 / np.maximum(cg, 1)
        in_maps.append({"h": h, "inv": inv})

    nc = _build(t_cap)
    try:
        res = run_bass_kernel_spmd(
            nc, in_maps, core_ids=list(range(NCORES)), trace=TRACE
        )
    except Exception:
        # transient device state (e.g. a previous run left a core wedged)
        # has been observed to clear on retry
        res = run_bass_kernel_spmd(
            nc, in_maps, core_ids=list(range(NCORES)), trace=TRACE
        )
    LAST_RESULTS = res

    out = np.concatenate([res.results[k]["out"] for k in range(NCORES)], axis=0)
    return out[:G]


# revision 10
# speedup vs baseline: 2.7839x; 1.5119x over previous
"""Trainium2 Bass kernel for batched global mean pooling (segment mean).

Computes, for N sorted nodes with 64 features and G graphs:
    out[g, f] = mean over nodes n with batch[n] == g of node_features[n, f]
(empty graphs -> zeros), distributed over 8 NeuronCores.

Strategy (graph sharding; no collectives):
  - Core k owns graphs [128k, 128(k+1)). batch is sorted, so each graph's
    nodes are a contiguous row range of node_features.
  - Host (inside kernel(), per call) lays out each core's nodes on a
    [128, T] grid: partition p gets only the nodes of local graph p,
    zero-padded to T = max graph size.
  - Features are sent as fp8 E3M4 (1 B/elt) with error-feedback
    quantization on the host: the rounding error of each node is carried
    into the next node of the same graph before quantizing, so the errors
    telescope in the segment sum and only the final carry (~one quantum)
    survives -> ~3e-4 rel error on the pooled means (2e-2 gate), at HALF
    the HBM traffic of fp16.
  - Device: each matmul is identity128.T @ slab for a [128, 8*64] fp8
    slab, accumulating into one [128, 512] f32 PSUM bank: partition =
    local graph. After all chunks: fold the 8 column blocks, multiply by
    host-provided 1/max(count, 1), DMA the [128, 64] result out.
  - Host concatenates the 8 per-core [128, 64] outputs.

The Bass program is compiled per call with the chunk count derived from
the actual input, so any node/graph distribution is handled.
"""

import math

import ml_dtypes
import numpy as np

import concourse.mybir as mybir
import concourse.tile as tile
from concourse import bacc
from concourse.bass_utils import run_bass_kernel_spmd
from concourse.masks import make_identity

NCORES = 8
P = 128  # partitions = local graphs per core
F = 64  # features
B = 8  # tiles (node-rows) per matmul: N = 8*64 = 512 f32 = one PSUM bank
TB = 64  # tiles per full DMA chunk (1 MB per chunk)

# set by tests to capture a profile; harness path leaves these alone
TRACE = False
LAST_RESULTS = None


def _chunks(t_cap):
    """Split t_cap tiles into DMA chunks: small 24-tile chunks at the START
    (so the first chunk lands quickly and the PE starts early instead of
    trailing the stream by the whole prefetch depth) and at the END (short PE
    tail after the final DMA); full 64-tile chunks in between."""
    out = []
    t = 0
    taper = TB if t_cap > 8 * TB else 0
    while t < t_cap:
        in_taper = t < taper or t_cap - t <= taper
        n = min(3 * B if in_taper else TB, t_cap - t)
        out.append((t, n))
        t += n
    return out


def _build(t_cap):
    nc = bacc.Bacc("TRN2", target_bir_lowering=False, debug=False, num_devices=NCORES)
    h = nc.dram_tensor(
        "h", [P, t_cap * F], mybir.dt.float8e3, kind="ExternalInput"
    ).ap()
    inv = nc.dram_tensor("inv", [P, 1], mybir.dt.float32, kind="ExternalInput").ap()
    out = nc.dram_tensor("out", [P, F], mybir.dt.float32, kind="ExternalOutput").ap()

    chunks = _chunks(t_cap)
    n_mm = t_cap // B
    with tile.TileContext(nc) as tc:
        with (
            tc.tile_pool(name="consts", bufs=1) as consts,
            tc.tile_pool(name="io", bufs=5) as io,
            tc.tile_pool(name="ep", bufs=1) as ep,
            tc.tile_pool(name="acc", bufs=1, space="PSUM") as accp,
        ):
            # build the identity on-device (GpSimd) so the weight preload has
            # no DMA dependency -- an identity DMA would queue behind the
            # first big chunk DMAs and stall the PE ~14 us at kernel start
            ident_sb = consts.tile([P, P], mybir.dt.float8e3)
            make_identity(nc, ident_sb[:])

            # load the identity into the PE array once; every matmul below
            # reuses it (ldweights=False) instead of reloading 128 columns
            # per matmul (~100 ns each)
            ldw = nc.tensor.ldweights(ident_sb[:])

            psum = accp.tile([P, B * F], mybir.dt.float32)
            mm = 0
            for ci, (t0, nt) in enumerate(chunks):
                h_t = io.tile([P, TB * F], mybir.dt.float8e3, tag="h")
                # alternate the two HWDGE rings (SP / ACT engines)
                eng = nc.sync if ci % 2 == 0 else nc.scalar
                eng.dma_start(h_t[:, : nt * F], h[:, t0 * F : (t0 + nt) * F])
                for b in range(nt // B):
                    inst = nc.tensor.matmul(
                        psum[:],
                        ident_sb[:],
                        h_t[:, b * B * F : (b + 1) * B * F],
                        start=(mm == 0),
                        stop=(mm == n_mm - 1),
                    )
                    inst.ins.ldweights = False
                    if mm == 0:
                        tile.add_dep_helper(
                            inst.ins,
                            ldw.ins,
                            sync=False,
                            reason="identity weights preloaded once",
                        )
                    mm += 1
            assert mm == n_mm

            # emitted after the chunk loop so this tiny transfer doesn't
            # head-of-line block the first chunk on the sync ring
            inv_sb = consts.tile([P, 1], mybir.dt.float32)
            nc.sync.dma_start(inv_sb[:], inv[:])

            # fold the B column blocks in ONE reduce: view psum [P, 512] as
            # [P, f=64, b=8] (b strided by 64) and sum the innermost axis
            s = ep.tile([P, F], mybir.dt.float32)
            nc.vector.tensor_reduce(
                s[:],
                psum[:, 0 : B * F].rearrange("p (b f) -> p f b", b=B),
                axis=mybir.AxisListType.X,
                op=mybir.AluOpType.add,
            )

            res = ep.tile([P, F], mybir.dt.float32)
            nc.vector.tensor_scalar_mul(res[:], s[:], inv_sb[:])
            nc.sync.dma_start(out[:], res[:])

    nc.compile()
    # bacc materializes one Ldweights per Matmult even with ldweights=False;
    # they all reload the same identity (~100 ns of PE time each). Drop the
    # redundant ones — keep any that carry semaphore waits/updates (those
    # park sync state), including the explicit preload which waits on the
    # identity build.
    for fn in nc.m.functions:
        for blk in fn.blocks:
            keep = [
                inst
                for inst in blk.instructions
                if not (
                    isinstance(inst, mybir.InstLdweights)
                    and (
                        inst.sync_info is None
                        or (
                            len(inst.sync_info.on_wait) == 0
                            and len(inst.sync_info.on_update) == 0
                        )
                    )
                )
            ]
            if len(keep) != len(blk.instructions):
                blk.instructions = keep
    return nc


def kernel(node_features, batch, num_graphs):
    global LAST_RESULTS
    x = np.asarray(node_features, dtype=np.float32)
    b = np.asarray(batch, dtype=np.int64).ravel()
    G = int(num_graphs)
    N = x.shape[0]
    assert x.shape[1] == F, f"expected {F} features, got {x.shape[1]}"

    if not np.all(b[1:] >= b[:-1]):  # defensive: layout relies on sorted batch
        order = np.argsort(b, kind="stable")
        b = b[order]
        x = x[order]

    gpc = math.ceil(G / NCORES)  # local graphs per core
    assert gpc <= P, f"num_graphs {G} too large for {NCORES} cores x {P} partitions"

    # ids >= G (if any) are dropped, matching segment_sum(num_segments=G)
    counts = np.bincount(b, minlength=NCORES * gpc)[: NCORES * gpc].astype(np.int64)
    starts = np.zeros(NCORES * gpc + 1, dtype=np.int64)
    np.cumsum(counts, out=starts[1:])
    t_max = int(counts.max()) if N else 1
    t_cap = max(B, math.ceil(t_max / B) * B)

    x_ext = np.vstack([x, np.zeros((1, F), dtype=np.float32)])  # row N = zeros
    col = np.arange(t_cap, dtype=np.int64)

    in_maps = []
    for k in range(NCORES):
        g0 = k * gpc
        cg = counts[g0 : g0 + gpc]
        sg = starts[g0 : g0 + gpc]
        cg_full = np.zeros(P, dtype=np.int64)
        cg_full[:gpc] = cg
        sg_full = np.full(P, N, dtype=np.int64)
        sg_full[:gpc] = sg
        valid = col[None, :] < cg_full[:, None]  # [P, t_cap]
        idx = np.where(valid, sg_full[:, None] + col[None, :], N)

        feats = x_ext[idx]  # [P, t_cap, F] f32
        # error-feedback quantization to fp8 E3M4 along the node axis: the
        # running carry makes the rounding errors telescope in the per-graph
        # sum, leaving only the final carry (~one quantum / count ~ 1e-5)
        q = np.empty((P, t_cap, F), dtype=ml_dtypes.float8_e3m4)
        carry = np.zeros((P, F), dtype=np.float32)
        for t in range(t_cap):
            y = feats[:, t, :] + carry
            qt = y.astype(ml_dtypes.float8_e3m4)
            q[:, t, :] = qt
            carry = y - qt.astype(np.float32)
        # zero the padding slots: the carry leaking past a graph's last node
        # would otherwise quantize to a spurious nonzero value there
        q[~valid] = 0
        h = q.reshape(P, t_cap * F)

        inv = np.zeros((P, 1), dtype=np.float32)
        inv[:gpc, 0] = 1.0 / np.maximum(cg, 1)
        in_maps.append({"h": h, "inv": inv})

    nc = _build(t_cap)
    try:
        res = run_bass_kernel_spmd(
            nc, in_maps, core_ids=list(range(NCORES)), trace=TRACE
        )
    except Exception:
        # transient device state (e.g. a previous run left a core wedged)
        # has been observed to clear on retry
        res = run_bass_kernel_spmd(
            nc, in_maps, core_ids=list(range(NCORES)), trace=TRACE
        )
    LAST_RESULTS = res

    out = np.concatenate([res.results[k]["out"] for k in range(NCORES)], axis=0)
    return out[:G]


# revision 11
# speedup vs baseline: 3.1276x; 1.1235x over previous
"""Trainium2 Bass kernel for batched global mean pooling (segment mean).

Computes, for N sorted nodes with 64 features and G graphs:
    out[g, f] = mean over nodes n with batch[n] == g of node_features[n, f]
(empty graphs -> zeros), distributed over 8 NeuronCores.

Strategy (graph sharding; no collectives):
  - Core k owns graphs [128k, 128(k+1)). batch is sorted, so each graph's
    nodes are a contiguous row range of node_features.
  - Host (inside kernel(), per call) lays out each core's nodes on a
    [128, T] grid: partition p gets only the nodes of local graph p,
    zero-padded to T = max graph size.
  - Features are sent as fp8 E4M3 (1 B/elt) with error-feedback
    quantization on the host: the rounding error of each node is carried
    into the next node of the same graph before quantizing, so the errors
    telescope in the segment sum and only the final carry (~one quantum)
    survives -> ~1e-3 rel error on the pooled means (2e-2 gate), at HALF
    the HBM traffic of fp16.
  - Device: DoubleRow matmuls. The stationary operand is [I | I] (the
    128x128 identity twice, viewed [128, 2, 128]), so each matmul
    consumes TWO 8-tile groups of the fp8 slab ([128, 2, 512] view) per
    pass and adds both into one [128, 512] f32 PSUM bank at 2 fp8/cell/
    cycle -- ~1.8x the plain-fp8 stream rate, dropping the PE below the
    DMA roofline. Partition = local graph. After all chunks: fold the 8
    column blocks, multiply by host-provided 1/max(count, 1), DMA the
    [128, 64] result out.
  - Host concatenates the 8 per-core [128, 64] outputs.

The Bass program is compiled per call with the chunk count derived from
the actual input, so any node/graph distribution is handled.
"""

import math

import ml_dtypes
import numpy as np

import concourse.mybir as mybir
import concourse.tile as tile
from concourse import bacc
from concourse.bass_utils import run_bass_kernel_spmd
from concourse.masks import make_identity

NCORES = 8
P = 128  # partitions = local graphs per core
F = 64  # features
B = 8  # tile-blocks resident in PSUM: 8*64 = 512 f32 = one PSUM bank
BMM = 16  # tiles consumed per DoubleRow matmul (two 8-tile groups)
TB = 128  # tiles per full DMA chunk (1 MB per chunk at 1 B/elt)

# set by tests to capture a profile; harness path leaves these alone
TRACE = False
LAST_RESULTS = None


def _chunks(t_cap):
    """Split t_cap tiles into DMA chunks: small 32-tile chunks at the START
    (so the first chunk lands quickly and the PE starts early instead of
    trailing the stream by the whole prefetch depth) and at the END (short PE
    tail after the final DMA); full 128-tile chunks in between."""
    out = []
    t = 0
    taper = TB if t_cap > 8 * TB else 0
    while t < t_cap:
        in_taper = t < taper or t_cap - t <= taper
        n = min(2 * BMM if in_taper else TB, t_cap - t)
        out.append((t, n))
        t += n
    return out


def _build(t_cap):
    nc = bacc.Bacc("TRN2", target_bir_lowering=False, debug=False, num_devices=NCORES)
    h = nc.dram_tensor(
        "h", [P, t_cap * F], mybir.dt.float8e4, kind="ExternalInput"
    ).ap()
    inv = nc.dram_tensor("inv", [P, 1], mybir.dt.float32, kind="ExternalInput").ap()
    out = nc.dram_tensor("out", [P, F], mybir.dt.float32, kind="ExternalOutput").ap()

    chunks = _chunks(t_cap)
    n_mm = t_cap // BMM
    with tile.TileContext(nc) as tc:
        with (
            tc.tile_pool(name="consts", bufs=1) as consts,
            tc.tile_pool(name="io", bufs=5) as io,
            tc.tile_pool(name="ep", bufs=1) as ep,
            tc.tile_pool(name="acc", bufs=1, space="PSUM") as accp,
        ):
            # build [I | I] on-device (GpSimd) so the weight preload has no
            # DMA dependency -- an identity DMA would queue behind the first
            # big chunk DMAs and stall the PE at kernel start
            ident2 = consts.tile([P, 2 * P], mybir.dt.float8e4)
            make_identity(nc, ident2[:, 0:P])
            make_identity(nc, ident2[:, P : 2 * P])
            w3 = ident2[:].rearrange("p (j g) -> p j g", j=2)

            # load both identity copies into the PE array once (DoubleRow
            # packs 2 fp8 weights per cell); every matmul below reuses them
            ldw = nc.tensor.ldweights(w3, perf_mode=mybir.MatmulPerfMode.DoubleRow)

            psum = accp.tile([P, B * F], mybir.dt.float32)
            mm = 0
            for ci, (t0, nt) in enumerate(chunks):
                h_t = io.tile([P, TB * F], mybir.dt.float8e4, tag="h")
                # alternate the two HWDGE rings (SP / ACT engines)
                eng = nc.sync if ci % 2 == 0 else nc.scalar
                eng.dma_start(h_t[:, : nt * F], h[:, t0 * F : (t0 + nt) * F])
                for b in range(nt // BMM):
                    rhs = h_t[:, b * BMM * F : (b + 1) * BMM * F].rearrange(
                        "p (j n) -> p j n", j=2
                    )
                    inst = nc.tensor.matmul(
                        psum[:],
                        w3,
                        rhs,
                        start=(mm == 0),
                        stop=(mm == n_mm - 1),
                        perf_mode=mybir.MatmulPerfMode.DoubleRow,
                    )
                    inst.ins.ldweights = False
                    if mm == 0:
                        tile.add_dep_helper(
                            inst.ins,
                            ldw.ins,
                            sync=False,
                            reason="identity weights preloaded once",
                        )
                    mm += 1
            assert mm == n_mm

            # emitted after the chunk loop so this tiny transfer doesn't
            # head-of-line block the first chunk on the sync ring
            inv_sb = consts.tile([P, 1], mybir.dt.float32)
            nc.sync.dma_start(inv_sb[:], inv[:])

            # fold the B column blocks in ONE reduce: view psum [P, 512] as
            # [P, f=64, b=8] (b strided by 64) and sum the innermost axis
            s = ep.tile([P, F], mybir.dt.float32)
            nc.vector.tensor_reduce(
                s[:],
                psum[:, 0 : B * F].rearrange("p (b f) -> p f b", b=B),
                axis=mybir.AxisListType.X,
                op=mybir.AluOpType.add,
            )

            res = ep.tile([P, F], mybir.dt.float32)
            nc.vector.tensor_scalar_mul(res[:], s[:], inv_sb[:])
            nc.sync.dma_start(out[:], res[:])

    nc.compile()
    # bacc materializes one Ldweights per Matmult even with ldweights=False;
    # they all reload the same identity (~100 ns of PE time each). Drop the
    # redundant ones — keep any that carry semaphore waits/updates (those
    # park sync state), including the explicit preload which waits on the
    # identity build.
    for fn in nc.m.functions:
        for blk in fn.blocks:
            keep = [
                inst
                for inst in blk.instructions
                if not (
                    isinstance(inst, mybir.InstLdweights)
                    and (
                        inst.sync_info is None
                        or (
                            len(inst.sync_info.on_wait) == 0
                            and len(inst.sync_info.on_update) == 0
                        )
                    )
                )
            ]
            if len(keep) != len(blk.instructions):
                blk.instructions = keep
    return nc


def kernel(node_features, batch, num_graphs):
    global LAST_RESULTS
    x = np.asarray(node_features, dtype=np.float32)
    b = np.asarray(batch, dtype=np.int64).ravel()
    G = int(num_graphs)
    N = x.shape[0]
    assert x.shape[1] == F, f"expected {F} features, got {x.shape[1]}"

    if not np.all(b[1:] >= b[:-1]):  # defensive: layout relies on sorted batch
        order = np.argsort(b, kind="stable")
        b = b[order]
        x = x[order]

    gpc = math.ceil(G / NCORES)  # local graphs per core
    assert gpc <= P, f"num_graphs {G} too large for {NCORES} cores x {P} partitions"

    # ids >= G (if any) are dropped, matching segment_sum(num_segments=G)
    counts = np.bincount(b, minlength=NCORES * gpc)[: NCORES * gpc].astype(np.int64)
    starts = np.zeros(NCORES * gpc + 1, dtype=np.int64)
    np.cumsum(counts, out=starts[1:])
    t_max = int(counts.max()) if N else 1
    t_cap = max(BMM, math.ceil(t_max / BMM) * BMM)

    x_ext = np.vstack([x, np.zeros((1, F), dtype=np.float32)])  # row N = zeros
    col = np.arange(t_cap, dtype=np.int64)

    # global [G_pad, t_cap] node-index grid (row N of x_ext = zeros), used to
    # run the error-feedback quantization vectorized across ALL graphs at
    # once (the carry chains run along the node axis, graphs are lanes)
    cg_all = counts
    sg_all = starts[:-1]
    valid_all = col[None, :] < cg_all[:, None]  # [G_pad, t_cap]
    idx_all = np.where(valid_all, sg_all[:, None] + col[None, :], N)

    # error-feedback quantization to fp8 E4M3 along the node axis: the
    # running carry makes the rounding errors telescope in the per-graph
    # sum, leaving only the final carry (~one quantum / count ~ 1e-5)
    G_pad = idx_all.shape[0]
    q_all = np.empty((G_pad, t_cap, F), dtype=ml_dtypes.float8_e4m3)
    carry = np.zeros((G_pad, F), dtype=np.float32)
    for t in range(t_cap):
        y = x_ext[idx_all[:, t]] + carry
        qt = y.astype(ml_dtypes.float8_e4m3)
        q_all[:, t, :] = qt
        carry = y - qt.astype(np.float32)
    # zero the padding slots: the carry leaking past a graph's last node
    # would otherwise quantize to a spurious nonzero value there
    q_all[~valid_all] = 0

    in_maps = []
    for k in range(NCORES):
        g0 = k * gpc
        qk = q_all[g0 : g0 + gpc]
        if gpc < P:
            qk = np.concatenate(
                [qk, np.zeros((P - gpc, t_cap, F), dtype=ml_dtypes.float8_e4m3)]
            )
        h = np.ascontiguousarray(qk).reshape(P, t_cap * F)

        inv = np.zeros((P, 1), dtype=np.float32)
        inv[:gpc, 0] = 1.0 / np.maximum(counts[g0 : g0 + gpc], 1)
        in_maps.append({"h": h, "inv": inv})

    nc = _build(t_cap)
    try:
        res = run_bass_kernel_spmd(
            nc, in_maps, core_ids=list(range(NCORES)), trace=TRACE
        )
    except Exception:
        # transient device state (e.g. a previous run left a core wedged)
        # has been observed to clear on retry
        res = run_bass_kernel_spmd(
            nc, in_maps, core_ids=list(range(NCORES)), trace=TRACE
        )
    LAST_RESULTS = res

    out = np.concatenate([res.results[k]["out"] for k in range(NCORES)], axis=0)
    return out[:G]


# revision 16
# speedup vs baseline: 3.1626x; 1.0112x over previous
"""Trainium2 Bass kernel for batched global mean pooling (segment mean).

Computes, for N sorted nodes with 64 features and G graphs:
    out[g, f] = mean over nodes n with batch[n] == g of node_features[n, f]
(empty graphs -> zeros), distributed over 8 NeuronCores.

Strategy (graph sharding; no collectives):
  - Core k owns graphs [128k, 128(k+1)). batch is sorted, so each graph's
    nodes are a contiguous row range of node_features.
  - Host (inside kernel(), per call) lays out each core's nodes on a
    [128, T] grid: partition p gets only the nodes of local graph p,
    zero-padded to T = max graph size.
  - Features are sent as fp8 E4M3 (1 B/elt) with error-feedback
    quantization on the host: the rounding error of each node is carried
    into the next node of the same graph before quantizing, so the errors
    telescope in the segment sum and only the final carry (~one quantum)
    survives -> ~1e-3 rel error on the pooled means (2e-2 gate), at HALF
    the HBM traffic of fp16.
  - Device: DoubleRow matmuls. The stationary operand is [I | I] (the
    128x128 identity twice, viewed [128, 2, 128]), so each matmul
    consumes TWO 8-tile groups of the fp8 slab ([128, 2, 512] view) per
    pass and adds both into one [128, 512] f32 PSUM bank at 2 fp8/cell/
    cycle -- ~1.8x the plain-fp8 stream rate, keeping the PE below the
    DMA roofline. Partition = local graph. After all chunks: fold the 8
    column blocks, multiply by host-provided 1/max(count, 1), DMA the
    [128, 64] result out.
  - Hand-rolled synchronization instead of TileContext: 6 semaphores and
    one HWDGE ring (SP). The Tile scheduler's fixed preamble + teardown
    (sem init, per-sem resets, all-engine barrier chains) cost ~17 us on
    a ~50 us kernel; the manual program's sync overhead is ~1 us.
  - Host concatenates the 8 per-core [128, 64] outputs.

The Bass program is compiled per call with the chunk count derived from
the actual input, so any node/graph distribution is handled.
"""

import math
from contextlib import ExitStack

import ml_dtypes
import numpy as np

import concourse.mybir as mybir
from concourse import bacc
from concourse.bass_utils import run_bass_kernel_spmd

NCORES = 8
P = 128  # partitions = local graphs per core
F = 64  # features
B = 8  # tile-blocks resident in PSUM: 8*64 = 512 f32 = one PSUM bank
BMM = 16  # tiles consumed per DoubleRow matmul (two 8-tile groups)
TB = 128  # tiles per full DMA chunk (1 MB per chunk at 1 B/elt)
NBUF = 8  # SBUF chunk ring depth (8 KB/partition each)

# set by tests to capture a profile; harness path leaves these alone
TRACE = False
LAST_RESULTS = None


def _chunks(t_cap):
    """Split t_cap tiles into DMA chunks: a couple of small 32-tile chunks at
    the START (first chunk lands quickly, PE starts early) and at the END
    (short PE tail after the final DMA); full 128-tile chunks in between."""
    out = []
    t = 0
    taper = 2 * BMM if t_cap > 4 * TB else 0
    while t < t_cap:
        in_taper = t < 2 * taper or t_cap - t <= 2 * taper
        n = min(2 * BMM if in_taper else TB, t_cap - t)
        out.append((t, n))
        t += n
    return out


def _ident(nc, ap, sq):
    """identity matrix into ap ([sq, sq]) on GpSimd, like masks.make_identity"""
    nc.gpsimd.memset(ap, 0.0)
    return nc.gpsimd.affine_select(
        out=ap,
        in_=ap,
        compare_op=mybir.AluOpType.not_equal,
        fill=1.0,
        base=0,
        pattern=[[-1, sq]],
        channel_multiplier=1,
    )


def _build(t_cap):
    nc = bacc.Bacc("TRN2", target_bir_lowering=False, debug=False, num_devices=NCORES)
    h = nc.dram_tensor(
        "h", [P, t_cap * F], mybir.dt.float8e4, kind="ExternalInput"
    ).ap()
    inv = nc.dram_tensor("inv", [P, 1], mybir.dt.float32, kind="ExternalInput").ap()
    out = nc.dram_tensor("out", [P, F], mybir.dt.float32, kind="ExternalOutput").ap()

    chunks = _chunks(t_cap)
    n_chunks = len(chunks)
    n_mm = t_cap // BMM

    # one completion lane per ring-buffer slot: HWDGE completions on a single
    # ring are NOT FIFO (the 16 SDMA engines interleave packets across queued
    # DMAs), so a single counting semaphore would let the PE start on a
    # partially-landed chunk. Lane ci%NBUF orders correctly because the NEXT
    # DMA on a lane is only issued after the previous one's chunk was fully
    # consumed (the s_mm buffer-reuse gate below).
    s_dma = [nc.alloc_semaphore(f"s_dma{i}") for i in range(NBUF)]
    s_inv = nc.alloc_semaphore("s_inv")  # inv DMA complete
    s_mm = nc.alloc_semaphore("s_mm")  # chunks fully consumed by PE
    s_idw = nc.alloc_semaphore("s_idw")  # identity built (GpSimd -> PE)
    s_done = nc.alloc_semaphore("s_done")  # PE drained, PSUM complete
    s_dve = nc.alloc_semaphore("s_dve")  # result ready in SBUF
    s_out = nc.alloc_semaphore("s_out")  # output DMA complete

    with ExitStack() as stk:
        t_id = stk.enter_context(nc.sbuf_tensor("ident2", [P, 2 * P], mybir.dt.float8e4))
        t_buf = stk.enter_context(
            nc.sbuf_tensor("bufs", [P, NBUF * TB * F], mybir.dt.float8e4)
        )
        t_inv = stk.enter_context(nc.sbuf_tensor("inv_sb", [P, 1], mybir.dt.float32))
        t_sum = stk.enter_context(nc.sbuf_tensor("ssum", [P, F], mybir.dt.float32))
        t_res = stk.enter_context(nc.sbuf_tensor("res", [P, F], mybir.dt.float32))
        t_acc = stk.enter_context(nc.psum_tensor("acc", [P, B * F], mybir.dt.float32))

        ident2 = t_id.ap()
        bufs = t_buf.ap()
        inv_sb = t_inv.ap()
        ssum = t_sum.ap()
        res = t_res.ap()
        acc = t_acc.ap()

        # [I | I] built on-device (GpSimd): no DMA dependency, so the weight
        # preload happens while the first chunks stream in
        _ident(nc, ident2[:, 0:P], P)
        _ident(nc, ident2[:, P : 2 * P], P).then_inc(s_idw, 1)

        w3 = ident2[:].rearrange("p (j g) -> p j g", j=2)

        # all chunk DMAs on the one SP HWDGE ring
        for ci, (t0, nt) in enumerate(chunks):
            if ci >= NBUF:  # ring reuse: wait until PE consumed chunk ci-NBUF
                nc.sync.wait_ge(s_mm, ci - NBUF + 1)
            slot = (ci % NBUF) * TB * F
            nc.sync.dma_start(
                bufs[:, slot : slot + nt * F], h[:, t0 * F : (t0 + nt) * F]
            ).then_inc(s_dma[ci % NBUF], 16)
        nc.sync.dma_start(inv_sb[:], inv[:]).then_inc(s_inv, 16)

        # PE: preload both identity copies once (DoubleRow packs 2 fp8
        # weights per cell); every matmul below reuses them
        nc.tensor.wait_ge(s_idw, 1)
        nc.tensor.ldweights(w3, perf_mode=mybir.MatmulPerfMode.DoubleRow)

        mm = 0
        for ci, (t0, nt) in enumerate(chunks):
            nc.tensor.wait_ge(s_dma[ci % NBUF], 16 * (ci // NBUF + 1))
            slot = (ci % NBUF) * TB * F
            last = None
            for b in range(nt // BMM):
                rhs = bufs[:, slot + b * BMM * F : slot + (b + 1) * BMM * F].rearrange(
                    "p (j n) -> p j n", j=2
                )
                inst = nc.tensor.matmul(
                    acc[:],
                    w3,
                    rhs,
                    start=(mm == 0),
                    stop=(mm == n_mm - 1),
                    perf_mode=mybir.MatmulPerfMode.DoubleRow,
                )
                inst.ins.ldweights = False
                last = inst
                mm += 1
            last.then_inc(s_mm, 1)  # matmul retired => chunk buffer free
        assert mm == n_mm
        # drain the PE pipeline so the PSUM writes are visible to the DVE
        nc.tensor.drain().then_inc(s_done, 1)

        # DVE tail: fold the B column blocks in ONE reduce -- view psum
        # [P, 512] as [P, f=64, b=8] (b strided by 64), sum innermost axis
        nc.vector.wait_ge(s_done, 1)
        nc.vector.tensor_reduce(
            ssum[:],
            acc[:, 0 : B * F].rearrange("p (b f) -> p f b", b=B),
            axis=mybir.AxisListType.X,
            op=mybir.AluOpType.add,
        )
        nc.vector.wait_ge(s_inv, 16)  # inv landed
        nc.vector.tensor_scalar_mul(res[:], ssum[:], inv_sb[:]).then_inc(s_dve, 1)

        nc.sync.wait_ge(s_dve, 1)
        nc.sync.dma_start(out[:], res[:]).then_inc(s_out, 16)
        nc.sync.wait_ge(s_out, 16)  # output landed before program end

    nc.compile()
    # bacc materializes one Ldweights per Matmult even with ldweights=False;
    # they all reload the same identity. Drop the redundant ones — keep any
    # that carry semaphore waits/updates, including the explicit preload.
    for fn in nc.m.functions:
        for blk in fn.blocks:
            keep = [
                inst
                for inst in blk.instructions
                if not (
                    isinstance(inst, mybir.InstLdweights)
                    and (
                        inst.sync_info is None
                        or (
                            len(inst.sync_info.on_wait) == 0
                            and len(inst.sync_info.on_update) == 0
                        )
                    )
                )
            ]
            if len(keep) != len(blk.instructions):
                blk.instructions = keep
    return nc


def kernel(node_features, batch, num_graphs):
    global LAST_RESULTS
    x = np.asarray(node_features, dtype=np.float32)
    b = np.asarray(batch, dtype=np.int64).ravel()
    G = int(num_graphs)
    N = x.shape[0]
    assert x.shape[1] == F, f"expected {F} features, got {x.shape[1]}"

    if not np.all(b[1:] >= b[:-1]):  # defensive: layout relies on sorted batch
        order = np.argsort(b, kind="stable")
        b = b[order]
        x = x[order]

    gpc = math.ceil(G / NCORES)  # local graphs per core
    assert gpc <= P, f"num_graphs {G} too large for {NCORES} cores x {P} partitions"

    # ids >= G (if any) are dropped, matching segment_sum(num_segments=G)
    counts = np.bincount(b, minlength=NCORES * gpc)[: NCORES * gpc].astype(np.int64)
    starts = np.zeros(NCORES * gpc + 1, dtype=np.int64)
    np.cumsum(counts, out=starts[1:])
    t_max = int(counts.max()) if N else 1
    t_cap = max(BMM, math.ceil(t_max / BMM) * BMM)

    x_ext = np.vstack([x, np.zeros((1, F), dtype=np.float32)])  # row N = zeros
    col = np.arange(t_cap, dtype=np.int64)

    # global [G_pad, t_cap] node-index grid (row N of x_ext = zeros), used to
    # run the error-feedback quantization vectorized across ALL graphs at
    # once (the carry chains run along the node axis, graphs are lanes)
    cg_all = counts
    sg_all = starts[:-1]
    valid_all = col[None, :] < cg_all[:, None]  # [G_pad, t_cap]
    idx_all = np.where(valid_all, sg_all[:, None] + col[None, :], N)

    # error-feedback quantization to fp8 E4M3 along the node axis: the
    # running carry makes the rounding errors telescope in the per-graph
    # sum, leaving only the final carry (~one quantum / count ~ 1e-5)
    G_pad = idx_all.shape[0]
    q_all = np.empty((G_pad, t_cap, F), dtype=ml_dtypes.float8_e4m3)
    carry = np.zeros((G_pad, F), dtype=np.float32)
    for t in range(t_cap):
        y = x_ext[idx_all[:, t]] + carry
        qt = y.astype(ml_dtypes.float8_e4m3)
        q_all[:, t, :] = qt
        carry = y - qt.astype(np.float32)
    # zero the padding slots: the carry leaking past a graph's last node
    # would otherwise quantize to a spurious nonzero value there
    q_all[~valid_all] = 0

    in_maps = []
    for k in range(NCORES):
        g0 = k * gpc
        qk = q_all[g0 : g0 + gpc]
        if gpc < P:
            qk = np.concatenate(
                [qk, np.zeros((P - gpc, t_cap, F), dtype=ml_dtypes.float8_e4m3)]
            )
        h = np.ascontiguousarray(qk).reshape(P, t_cap * F)

        inv = np.zeros((P, 1), dtype=np.float32)
        inv[:gpc, 0] = 1.0 / np.maximum(counts[g0 : g0 + gpc], 1)
        in_maps.append({"h": h, "inv": inv})

    nc = _build(t_cap)
    try:
        res = run_bass_kernel_spmd(
            nc, in_maps, core_ids=list(range(NCORES)), trace=TRACE
        )
    except Exception:
        # transient device state (e.g. a previous run left a core wedged)
        # has been observed to clear on retry
        res = run_bass_kernel_spmd(
            nc, in_maps, core_ids=list(range(NCORES)), trace=TRACE
        )
    LAST_RESULTS = res

    out = np.concatenate([res.results[k]["out"] for k in range(NCORES)], axis=0)
    return out[:G]
